# revision 13
# baseline (speedup 1.0000x reference)
"""Trainium2 Bass kernel for nn_Attention_26182120636812 (GQA attention block).

Sharding: 8 cores = 2 (batch) x 4 (KV groups). Each core computes, for its
batch element b and kv-group g: the 4 query heads + 1 kv head of group g,
full causal attention over T=2048, and the partial output projection
y_part = o_g @ wo[g*512:(g+1)*512, :]. The host sums the 4 partials per batch.

v2 design (vs the f32r baseline):
 - all matmul operands are bf16 (same 1 cyc/row PE rate as f32r, but 1 cyc/row
   transposes, no N<256 f32r penalty, half the DMA/SBUF footprint, 2x DVE).
   PSUM accumulation stays fp32; scores, rstd, denominators in fp32.
 - softmax denominators come OFF the PE: att chunks are accumulated on DVE
   (bf16 chain), the [128,512] accumulator is PE-transposed so the reciprocal
   runs partition-parallel on [128,4] (the baseline's [1,512] DVE reciprocal
   was element-serial, ~3.4us, and stalled the PE queue 16x per pass).
 - output projection is fused into the attention pass: C-quads for query
   super-tile i-1 are interleaved between attention chunks of super-tile i,
   so the PE stays busy while ACT runs exp, and the HAM clock gate never
   re-throttles the PE to 1.2 GHz (the baseline lost ~70us/rep to cold-clock
   matmuls after >3.4us PE idle gaps).
 - stream finalize (transpose-reduce + reciprocal + broadcast + normalize) is
   software-pipelined one stream behind the chunk loop.
"""
import sys

for _p in ("/opt/trn_rl_repo",):
    if _p not in sys.path:
        sys.path.insert(0, _p)

import numpy as np

B, T, D = 2, 2048, 2048
H, KV, HD = 16, 4, 128
NCORES = 8
NH = H // KV          # 4 q heads per core
GW = NH * HD          # 512: per-core q / o width
TT = T // 128         # 16 token tiles
NI = T // 512         # 4 query super-tiles
DC = D // 128         # 16 contraction chunks over D
EPS = 1e-6
SCALE = 1.0 / float(np.sqrt(HD))
NEG = -1.0e30

# diagonal chunk r in 0..3 of a 512-wide query tile: computed tq offset/width
DIAG_OFF = [0, 128, 256, 256]
DIAG_W = [512, 384, 256, 256]

_prog_cache = {}


def _build(shared_freqs: bool, repeat: int = 1, timing: bool = False,
           unit_w: bool = True):
    import concourse.bacc as bacc
    import concourse.mybir as mybir
    import concourse.tile as tile

    dt = mybir.dt
    f32 = dt.float32
    bf = dt.bfloat16
    AF = mybir.ActivationFunctionType

    nc = bacc.Bacc("TRN2", target_bir_lowering=False, debug=False,
                   num_devices=NCORES)
    ikind = "Internal" if timing else "ExternalInput"
    okind = "Internal" if timing else "ExternalOutput"
    xT_d = nc.dram_tensor("xT", [D, T], bf, kind=ikind).ap()
    wq_d = nc.dram_tensor("wq", [D, GW], bf, kind=ikind).ap()
    wkv_d = nc.dram_tensor("wkv", [D, 2 * HD], bf, kind=ikind).ap()
    wo_d = nc.dram_tensor("wo", [GW, D], bf, kind=ikind).ap()
    nf = 256 if shared_freqs else 512
    f8_d = nc.dram_tensor("f8", [T, nf], f32, kind=ikind).ap()
    msk_d = nc.dram_tensor("msk", [128, 128], f32, kind=ikind).ap()
    eye_d = nc.dram_tensor("eye", [128, 128], bf, kind=ikind).ap()
    onr_d = nc.dram_tensor("onr", [1, 128], bf, kind=ikind).ap()
    y_d = nc.dram_tensor("y", [T, D], f32, kind=okind).ap()
    if timing:
        din = nc.dram_tensor("din", [128, 4], f32, kind="ExternalInput").ap()
        dout = nc.dram_tensor("dout", [128, 4], f32,
                              kind="ExternalOutput").ap()

    def mm(out, lhsT, rhs, start, stop):
        nc.tensor.matmul(out, lhsT, rhs, start=start, stop=stop,
                         skip_group_check=True)

    with nc.allow_low_precision(reason="bf16 operands feed fp32-accum "
                                "matmuls; tolerance is 2e-2"), \
         tile.TileContext(nc) as tc:
        with tc.tile_pool(name="const", bufs=1) as cpool, \
             tc.tile_pool(name="resid", bufs=1) as rpool:
            if timing:
                dsb = cpool.tile([128, 4], f32)
                nc.sync.dma_start(dsb[:], din[:])
                nc.sync.dma_start(dout[:], dsb[:])
            msk_sb = cpool.tile([128, 128], f32)
            nc.sync.dma_start(msk_sb[:], msk_d[:])
            eye_sb = cpool.tile([128, 128], bf)
            nc.sync.dma_start(eye_sb[:], eye_d[:])
            onr_sb = cpool.tile([1, 128], bf)
            nc.sync.dma_start(onr_sb[:], onr_d[:])
            eps_sb = cpool.tile([128, 1], f32)
            nc.vector.memset(eps_sb[:], EPS)

            # residents: qT/kT head_dim-major, v natural, oT 2-buf ping-pong
            # over super-tiles, wo resident for the fused output projection
            qT = rpool.tile([128, NH * T], bf)     # head h at [:, h*T:(h+1)*T]
            kT = rpool.tile([128, T], bf)
            vv = rpool.tile([128, T], bf)          # chunk j at [:, j*128:...]
            oT = rpool.tile([128, 2 * NH * 512], bf)  # [buf][h] slots
            wo_sb = rpool.tile([128, NH * NI * 512], bf)  # (lc,d) slot

            def make_quad_emitter(psY, ysb):
                ycnt = [0]

                def emit_quad(iq, tq, dq):
                    # y tile (token tile tq, d-chunk dq) from super-tile iq
                    buf = iq % 2
                    tg = iq * 4 + tq
                    y_ps = psY.tile([128, 512], f32, name="y_ps")
                    for lc in range(NH):
                        ot = oT[:, (buf * NH + lc) * 512 + tq * 128:
                                (buf * NH + lc) * 512 + (tq + 1) * 128]
                        mm(y_ps[:], ot, wo_sb[:, (lc * NI + dq) * 512:
                                              (lc * NI + dq + 1) * 512],
                           lc == 0, lc == NH - 1)
                    y_sb = ysb.tile([128, 512], f32, name="y_sb")
                    if ycnt[0] % 2 == 0:
                        nc.scalar.copy(y_sb[:], y_ps[:])
                    else:
                        nc.vector.tensor_copy(y_sb[:], y_ps[:])
                    ycnt[0] += 1
                    nc.sync.dma_start(
                        y_d[tg * 128:(tg + 1) * 128,
                            dq * 512:(dq + 1) * 512], y_sb[:])

                return emit_quad

            pend_tail = []
            for _rep in range(repeat):
                # ---------------- phase A: projections + rmsnorm + rope -----
                with tc.tile_pool(name=f"wA{_rep}", bufs=1) as wA, \
                     tc.tile_pool(name=f"xs{_rep}", bufs=2) as xsp, \
                     tc.tile_pool(name=f"fA{_rep}", bufs=2) as fap, \
                     tc.tile_pool(name=f"qrp{_rep}", bufs=2) as qrp, \
                     tc.tile_pool(name=f"smA{_rep}", bufs=2) as smp, \
                     tc.tile_pool(name=f"ysA{_rep}", bufs=3) as ysbA, \
                     tc.tile_pool(name=f"psA{_rep}", bufs=2, space="PSUM") as psA, \
                     tc.tile_pool(name=f"psK{_rep}", bufs=2, space="PSUM") as psK, \
                     tc.tile_pool(name=f"psT{_rep}", bufs=2, space="PSUM") as psT, \
                     tc.tile_pool(name=f"psYA{_rep}", bufs=2, space="PSUM") as psYA:
                    wq_sb = wA.tile([128, DC * GW], bf)
                    wkv_sb = wA.tile([128, DC * 2 * HD], bf)
                    wqr = wq_sb.rearrange("p (c n) -> p c n", c=DC)
                    wqs = wq_d.rearrange("(c p) n -> p c n", p=128)
                    wkr = wkv_sb.rearrange("p (c n) -> p c n", c=DC)
                    wks = wkv_d.rearrange("(c p) n -> p c n", p=128)
                    for c in range(DC):
                        nc.sync.dma_start(wqr[:, c, :], wqs[:, c, :])
                        nc.sync.dma_start(wkr[:, c, :], wks[:, c, :])
                    wq_v = wq_sb.rearrange("p (c n) -> p c n", c=DC)
                    wkv_v = wkv_sb.rearrange("p (c n) -> p c n", c=DC)
                    # wo prefetch: lands during phase A, used by fused C
                    # (spread across ttiles so it never delays the x stream)
                    wor = wo_sb.rearrange("p (c n) -> p c n", c=NH * NI)
                    wos = wo_d.rearrange("(c p) n -> p c n", p=128)
                    wo_chunks = [(lc, dd) for lc in range(NH)
                                 for dd in range(NI)]

                    emitA = make_quad_emitter(psYA, ysbA)
                    xs = None
                    pend_qr = None
                    for t in range(TT):
                        while pend_tail and len(pend_tail) > 2 * (TT - 1 - t):
                            emitA(*pend_tail.pop(0))
                        g2, half = divmod(t, 2)
                        if half == 0:
                            xs = xsp.tile([128, DC * 256], bf, name="xs")
                            xsr = xs.rearrange("p (c n) -> p c n", c=DC)
                            xss = xT_d.rearrange("(c p) n -> p c n", p=128)
                            for c in range(DC):
                                nc.sync.dma_start(
                                    xsr[:, c, :],
                                    xss[:, c, g2 * 256:(g2 + 1) * 256])
                        xs_v = xs.rearrange("p (c n) -> p c n", c=DC)
                        f8t = fap.tile([128, nf], f32, name="f8t")
                        nc.sync.dma_start(f8t[:], f8_d[t * 128:(t + 1) * 128, :])
                        if 1 <= t <= 8 and wo_chunks:
                            for _ in range(2):
                                lc, dd = wo_chunks.pop()
                                nc.sync.dma_start(
                                    wor[:, lc * NI + dd, :],
                                    wos[:, lc, dd * 512:(dd + 1) * 512])

                        q_ps = psA.tile([128, GW], f32, name="q_ps")
                        kv_ps = psK.tile([128, 2 * HD], f32, name="kv_ps")
                        for c in range(DC):
                            xc = xs_v[:, c, half * 128:(half + 1) * 128]
                            mm(q_ps[:], xc, wq_v[:, c, :], c == 0, c == DC - 1)
                            mm(kv_ps[:], xc, wkv_v[:, c, :], c == 0,
                               c == DC - 1)

                        # v: natural layout, straight into the resident tile
                        nc.scalar.copy(vv[:, t * 128:(t + 1) * 128],
                                       kv_ps[:, HD:2 * HD])

                        ssq = smp.tile([128, 8], f32, name="ssq")
                        if not unit_w:
                            sqs = smp.tile([128, 128], f32, name="sqs")
                            for h5 in range(5):
                                src = (q_ps[:, h5 * 128:(h5 + 1) * 128]
                                       if h5 < 4 else kv_ps[:, 0:HD])
                                nc.scalar.activation(sqs[:], src, AF.Square,
                                                     accum_out=ssq[:,
                                                                   h5:h5 + 1])

                        # rope (freqs carry the rmsnorm weights); qr holds the
                        # 4 q heads then k, [evens|odds] within each 128 block
                        qr = qrp.tile([128, 640], bf, name="qr")
                        t1 = qrp.tile([128, 256], f32, name="t1")
                        t2 = qrp.tile([128, 256], f32, name="t2")
                        q_v = q_ps.rearrange("p (h x) -> p h x", h=4)
                        qe, qo = q_v[:, :, 0:64], q_v[:, :, 64:128]
                        t1_v = t1.rearrange("p (h x) -> p h x", h=4)
                        t2_v = t2.rearrange("p (h x) -> p h x", h=4)
                        qr_v = qr.rearrange("p (h x) -> p h x", h=5)

                        def fq(k4):  # freq slice broadcast over the 4 q heads
                            s = f8t[:, k4 * 64:(k4 + 1) * 64]
                            return s.rearrange("p (o x) -> p o x", o=1) \
                                    .broadcast_to([128, 4, 64])

                        nc.vector.tensor_mul(t1_v, qe, fq(0))          # e*cosE
                        nc.vector.tensor_mul(t2_v, qo, fq(1))          # o*sinO
                        nc.vector.tensor_sub(qr_v[:, 0:4, 0:64], t1_v, t2_v)
                        nc.vector.tensor_mul(t1_v, qe, fq(2))          # e*sinE
                        nc.vector.tensor_mul(t2_v, qo, fq(3))          # o*cosO
                        nc.vector.tensor_add(qr_v[:, 0:4, 64:128], t1_v, t2_v)

                        kf0 = 0 if shared_freqs else 4
                        ke, ko = kv_ps[:, 0:64], kv_ps[:, 64:128]
                        kt1 = smp.tile([128, 64], f32, name="kt1")
                        kt2 = smp.tile([128, 64], f32, name="kt2")

                        def fk(k4):
                            return f8t[:, (kf0 + k4) * 64:(kf0 + k4 + 1) * 64]

                        nc.vector.tensor_mul(kt1[:], ke, fk(0))
                        nc.vector.tensor_mul(kt2[:], ko, fk(1))
                        nc.vector.tensor_sub(qr[:, 512:576], kt1[:], kt2[:])
                        nc.vector.tensor_mul(kt1[:], ke, fk(2))
                        nc.vector.tensor_mul(kt2[:], ko, fk(3))
                        nc.vector.tensor_add(qr[:, 576:640], kt1[:], kt2[:])

                        if unit_w:
                            # rope is a pure rotation: per-head sum of squares
                            # can be taken from the (bf16) rope output
                            sqs = smp.tile([128, 128], bf, name="sqs")
                            for h5 in range(5):
                                sl = qr[:, h5 * 128:(h5 + 1) * 128]
                                nc.vector.scalar_tensor_tensor(
                                    sqs[:], sl, 1.0, sl,
                                    mybir.AluOpType.mult,
                                    mybir.AluOpType.mult,
                                    accum_out=ssq[:, h5:h5 + 1])
                        # rms = (mean+eps)^-0.5 as exp(-0.5*ln(.)): ln and exp
                        # share one ACT table, so the whole kernel runs off
                        # 'natural_log_exp_and_others' with zero mid-pass
                        # table reloads (Sqrt lives in a different table)
                        lnv = smp.tile([128, 8], f32, name="lnv")
                        nc.scalar.activation(lnv[:, 0:5], ssq[:, 0:5], AF.Ln,
                                             bias=eps_sb[:], scale=1.0 / HD)
                        rms = smp.tile([128, 8], f32, name="rms")
                        nc.scalar.activation(rms[:, 0:5], lnv[:, 0:5], AF.Exp,
                                             scale=-0.5)
                        for h5 in range(5):
                            sl = qr[:, h5 * 128:(h5 + 1) * 128]
                            nc.vector.tensor_scalar_mul(sl, sl, rms[:, h5:h5 + 1])

                        # transpose each head block into the resident qT / kT
                        # -- deferred one ttile (software pipeline) so the PE
                        # never waits on the DVE rope chain
                        if pend_qr is not None:
                            pqr, pt = pend_qr
                            for h5 in range(5):
                                tp_ps = psT.tile([128, 128], bf, name="tp_ps")
                                nc.tensor.transpose(
                                    tp_ps[:], pqr[:, h5 * 128:(h5 + 1) * 128],
                                    eye_sb[:])
                                dst = (qT[:, h5 * T + pt * 128:
                                          h5 * T + (pt + 1) * 128]
                                       if h5 < 4
                                       else kT[:, pt * 128:(pt + 1) * 128])
                                nc.scalar.copy(dst, tp_ps[:])
                        pend_qr = (qr, t)

                    pqr, pt = pend_qr
                    for h5 in range(5):
                        tp_ps = psT.tile([128, 128], bf, name="tp_ps")
                        nc.tensor.transpose(
                            tp_ps[:], pqr[:, h5 * 128:(h5 + 1) * 128],
                            eye_sb[:])
                        dst = (qT[:, h5 * T + pt * 128:
                                  h5 * T + (pt + 1) * 128]
                               if h5 < 4 else kT[:, pt * 128:(pt + 1) * 128])
                        nc.scalar.copy(dst, tp_ps[:])

                # ---------------- phase B+C: attention with fused out-proj --
                # PSUM banks: psS 2 (scores / finalize-transposes), psO 2
                # (o accum), psB 2 (1/d broadcast), psY 2 (y out) = 8.
                with tc.tile_pool(name=f"attp{_rep}", bufs=4) as attp, \
                     tc.tile_pool(name=f"accp{_rep}", bufs=2) as accp, \
                     tc.tile_pool(name=f"smB{_rep}", bufs=2) as smB, \
                     tc.tile_pool(name=f"ysb{_rep}", bufs=4) as ysb, \
                     tc.tile_pool(name=f"psS{_rep}", bufs=2, space="PSUM") as psS, \
                     tc.tile_pool(name=f"psF{_rep}", bufs=1, space="PSUM") as psF, \
                     tc.tile_pool(name=f"psO{_rep}", bufs=2, space="PSUM") as psO, \
                     tc.tile_pool(name=f"psB{_rep}", bufs=1, space="PSUM") as psB, \
                     tc.tile_pool(name=f"psY{_rep}", bufs=2, space="PSUM") as psY:

                    emit_quad = make_quad_emitter(psY, ysb)

                    # finalize stream (h,i): denominator transpose-reduce,
                    # partition-parallel reciprocal, broadcast, normalize oT
                    def fin_a(pf):
                        acc, h, i = pf
                        ft = psF.tile([128, 1024], bf, name="ft")
                        dT = smB.tile([128, 4], f32, name="dT")
                        dsc = smB.tile([128, 128], bf, name="dsc")
                        for s in range(4):
                            nc.tensor.transpose(
                                ft[:, s * 128:(s + 1) * 128],
                                acc[:, s * 128:(s + 1) * 128], eye_sb[:])
                            nc.vector.tensor_scalar(
                                dsc[:], ft[:, s * 128:(s + 1) * 128], 1.0,
                                0.0, mybir.AluOpType.mult,
                                mybir.AluOpType.add,
                                accum_out=dT[:, s:s + 1])
                        recT = smB.tile([128, 4], bf, name="recT")
                        nc.vector.reciprocal(recT[:], dT[:])
                        for s in range(4):
                            nc.tensor.transpose(
                                ft[0:1, 512 + s * 128:512 + (s + 1) * 128],
                                recT[:, s:s + 1], eye_sb[:])
                        rec_sb = smB.tile([1, 512], bf, name="rec_sb")
                        nc.scalar.copy(rec_sb[:], ft[0:1, 512:1024])
                        return rec_sb

                    def fin_b(pf, rec_sb):
                        o_ps, h, i = pf
                        buf = i % 2
                        bc_ps = psB.tile([128, 512], f32, name="bc_ps")
                        mm(bc_ps[:], onr_sb[:], rec_sb[0:1, :], True, True)
                        osl = oT[:, (buf * NH + h) * 512:
                                 (buf * NH + h + 1) * 512]
                        nc.scalar.copy(osl, o_ps[:])
                        nc.vector.tensor_mul(osl, osl, bc_ps[:])

                    pend_att = None   # (att, off, w, aslice, j, o_ps, acc)
                    pend_fa = None    # waiting for fin_a
                    pend_fb = None    # waiting for fin_b

                    def flush_pend_att(last):
                        nonlocal pend_att
                        if pend_att is None:
                            return
                        patt, poff, pw, pas, pj, po, pacc = pend_att
                        mm(po[:, poff:poff + pw], vv[:, pj * 128:(pj + 1) * 128],
                           patt[:, pas:pas + pw], pj == 0, last)
                        # denominator accumulation on DVE (bf16 chain)
                        if pj == 0:
                            nc.vector.tensor_copy(pacc[:], patt[:, 0:512])
                        else:
                            nc.vector.tensor_add(
                                pacc[:, poff:poff + pw],
                                pacc[:, poff:poff + pw], patt[:, pas:pas + pw])
                        pend_att = None

                    for i in range(NI):
                        # C-quads of the previous super-tile, interleaved
                        quads = ([(i - 1, tq, dq) for tq in range(4)
                                  for dq in range(NI)] if i > 0 else [])
                        slots = NH * (4 * i + 4)
                        qper = max(1, -(-len(quads) // max(1, slots - 4)))
                        qidx = [0]
                        slot = [0]

                        def tick():
                            # called once per chunk: drip C work into the
                            # PE stream so it never idles while ACT runs exp
                            if slot[0] >= 4:
                                n = 0
                                while (qidx[0] < len(quads) and n < qper):
                                    emit_quad(*quads[qidx[0]])
                                    qidx[0] += 1
                                    n += 1
                            slot[0] += 1

                        for h in range(NH):
                            o_ps = psO.tile([128, 512], f32, name="o_ps")
                            acc = accp.tile([128, 512], bf, name="acc")
                            nj = 4 * i + 4
                            rec_hold = [None]
                            for j in range(nj):
                                if j == 1 and pend_fa is not None:
                                    rec_hold[0] = fin_a(pend_fa[0])
                                    pend_fb = pend_fa[1]
                                    pend_fa = None
                                if j == 3 and pend_fb is not None:
                                    fin_b(pend_fb, rec_hold[0])
                                    pend_fb = None
                                r = j - 4 * i
                                off = DIAG_OFF[r] if r >= 0 else 0
                                w = DIAG_W[r] if r >= 0 else 512
                                s_ps = psS.tile([128, 512], f32, name="s_ps")
                                mm(s_ps[:, 0:w], kT[:, j * 128:(j + 1) * 128],
                                   qT[:, h * T + i * 512 + off:
                                       h * T + i * 512 + off + w], True, True)
                                att = attp.tile([128, 512], bf, name="att")
                                if r >= 0:
                                    mr = 128 if r == 3 else 0
                                    nc.vector.tensor_add(
                                        s_ps[:, mr:mr + 128],
                                        s_ps[:, mr:mr + 128], msk_sb[:])
                                if r == 3:
                                    # only [128:256] of this chunk is live;
                                    # shrink exp/AV/acc to it
                                    nc.scalar.activation(
                                        att[:, 128:256], s_ps[:, 128:256],
                                        AF.Exp, scale=SCALE)
                                    cur = (att, 384, 128, 128, j, o_ps, acc)
                                else:
                                    nc.scalar.activation(
                                        att[:, 0:w], s_ps[:, 0:w],
                                        AF.Exp, scale=SCALE)
                                    cur = (att, off, w, 0, j, o_ps, acc)
                                flush_pend_att(False)
                                pend_att = cur
                                tick()
                            flush_pend_att(True)
                            pend_fa = ((acc, h, i), (o_ps, h, i))
                        # drain any unemitted quads for this super-tile
                        while qidx[0] < len(quads):
                            emit_quad(*quads[qidx[0]])
                            qidx[0] += 1

                    # last stream finalize; its C-quads are deferred into
                    # the next rep's phase A (keeps PE busy across the rep
                    # boundary) except on the final rep
                    rec = fin_a(pend_fa[0])
                    fin_b(pend_fa[1], rec)
                    pend_fa = None
                    if _rep < repeat - 1:
                        pend_tail = [(NI - 1, tq, dq) for tq in range(4)
                                     for dq in range(NI)]
                    else:
                        for tq in range(4):
                            for dq in range(NI):
                                emit_quad(NI - 1, tq, dq)

    nc.compile()
    return nc


def prepare_inputs(x, wq, wk, wv, wo, q_norm_w, k_norm_w, freqs_cos, freqs_sin):
    """Host-side sharding + layout prep. Returns (in_maps, shared, unit_w)."""
    import ml_dtypes
    bfnp = ml_dtypes.bfloat16

    x = np.asarray(x, np.float32)
    wq = np.asarray(wq, np.float32)
    wk = np.asarray(wk, np.float32)
    wv = np.asarray(wv, np.float32)
    wo = np.asarray(wo, np.float32)
    qw = np.asarray(q_norm_w, np.float32)
    kw = np.asarray(k_norm_w, np.float32)
    cos = np.asarray(freqs_cos, np.float32)
    sin = np.asarray(freqs_sin, np.float32)

    perm = np.concatenate([np.arange(0, HD, 2), np.arange(1, HD, 2)])
    shared = bool(np.allclose(qw, kw))
    unit_w = bool(np.allclose(qw, 1.0) and np.allclose(kw, 1.0))

    def freq4(w):
        we, wo_ = w[0::2], w[1::2]
        return np.concatenate(
            [cos * we[None, :], sin * wo_[None, :],
             sin * we[None, :], cos * wo_[None, :]], axis=1)

    f8 = freq4(qw) if shared else np.concatenate([freq4(qw), freq4(kw)], axis=1)
    f8 = np.ascontiguousarray(f8, np.float32)

    msk = np.where(np.arange(128)[None, :] >= np.arange(128)[:, None],
                   np.float32(0.0), np.float32(NEG)).astype(np.float32)
    eye = np.eye(128, dtype=np.float32).astype(bfnp)
    onr = np.ones((1, 128), np.float32).astype(bfnp)

    xTs = [np.ascontiguousarray(x[b].T).astype(bfnp) for b in range(B)]
    in_maps = []
    for c in range(NCORES):
        b, g = divmod(c, KV)
        wq_g = wq[:, g * GW:(g + 1) * GW].reshape(D, NH, HD)[:, :, perm] \
            .reshape(D, GW)
        wk_g = wk[:, g * HD:(g + 1) * HD][:, perm]
        wv_g = wv[:, g * HD:(g + 1) * HD]
        wkv_g = np.ascontiguousarray(
            np.concatenate([wk_g, wv_g], axis=1)).astype(bfnp)
        wo_g = np.ascontiguousarray(wo[g * GW:(g + 1) * GW, :]).astype(bfnp)
        in_maps.append(dict(
            xT=xTs[b], wq=np.ascontiguousarray(wq_g).astype(bfnp),
            wkv=wkv_g, wo=wo_g, f8=f8, msk=msk, eye=eye, onr=onr))
    return in_maps, shared, unit_w


def get_program(shared_freqs: bool, repeat: int = 1, timing: bool = False,
                unit_w: bool = True):
    key = (shared_freqs, repeat, timing, unit_w)
    if key not in _prog_cache:
        _prog_cache[key] = _build(shared_freqs, repeat, timing, unit_w)
    return _prog_cache[key]


def kernel(**inputs):
    from concourse.bass_utils import run_bass_kernel_spmd

    in_maps, shared, unit_w = prepare_inputs(**inputs)
    nc = get_program(shared, unit_w=unit_w)
    res = run_bass_kernel_spmd(nc, in_maps, list(range(NCORES)))
    out = np.empty((B, T, D), np.float32)
    for b in range(B):
        acc = res.results[b * KV + 0]["y"].astype(np.float32)
        for g in range(1, KV):
            acc = acc + res.results[b * KV + g]["y"]
        out[b] = acc
    return out
